# revision 1
# baseline (speedup 1.0000x reference)
"""DelayAttention Trainium2 kernel (v3).

Data-parallel over batch: B=16 split as 2 batches per core across 8 cores.
Per core, per batch, the sequence is processed in 512-token slices
(tokens = (t, n) pairs, 64 nodes per timestep):

  1. x is pre-cast to bf16 on host; DMA-transpose loads xT [d, tok] directly
     (no PE transposes for x).
  2. Linears Q/K/V/u as bf16 matmuls (weights stationary, K-chunked over d),
     outputs live transposed [dk, tok].
  3. sim[p, tok] via 10 accumulated bf16 matmuls (M=8) against a sliding
     window of the persistent bf16 UT buffer.
  4. pattern softmax, denominator-free: e = exp(sim); N = c_sum^T @ e
     (unnormalized injection); denominator computed TRANSPOSED as
     denT[tok] = e_chunk^T @ ones via 4 tiny matmuls -> one [128,4]
     reciprocal rd.
  5. attention with TRANSPOSED scores [k, q]: sc1 = K^T Q, sc2 = N^T Q,
     scores = sc1 + rd[k] * sc2 (rd is per-partition). exp without max
     (scores bounded); denominator via an extra ones-column appended to V
     so the AV matmul yields [out | rowsum]; final scale by 1/rowsum.
  PSUM (8 banks): lin x2, pat (sim/nt/den4) x2, att (sc1|sc2|o per pair,
  column regions of one bank) x2, vx (V-transpose) x2.
"""

import os
import sys

import numpy as np

for _p in ("/opt/trn_rl_repo",):
    if _p not in sys.path and os.path.isdir(_p):
        sys.path.insert(0, _p)

import ml_dtypes  # noqa: E402

import concourse.bass as bass  # noqa: E402
import concourse.mybir as mybir  # noqa: E402
import concourse.tile as tile  # noqa: E402
from concourse import bacc  # noqa: E402

F32 = mybir.dt.float32
BF16 = mybir.dt.bfloat16
AX = mybir.AxisListType.X
AF = mybir.ActivationFunctionType
ALU = mybir.AluOpType

N_CORES = 8
N_NODES = 64          # N
D_MODEL = 256         # D
DK = 128
S_WIN = 10            # window size
N_PAT = 8             # patterns
SL = 512              # tokens per slice
INJ0 = S_WIN * N_NODES  # 640: first injected token


def build_program(Bs: int, T: int) -> bass.Bass:
    TOK = T * N_NODES
    nsl = TOK // SL
    assert TOK % SL == 0
    scale = 1.0 / float(np.sqrt(DK))

    nc = bacc.Bacc("TRN2", target_bir_lowering=False, debug=False)

    x_in = nc.dram_tensor("x", [Bs, T, N_NODES, D_MODEL], BF16, kind="ExternalInput")
    wts = {
        k: nc.dram_tensor(f"wt{k}", [2, 128, DK], BF16, kind="ExternalInput")
        for k in ("q", "k", "v", "u")
    }
    biases_in = {
        k: nc.dram_tensor(f"b{k}", [DK, 1], F32, kind="ExternalInput")
        for k in ("q", "k", "v", "u")
    }
    mT_in = nc.dram_tensor("mT", [DK, S_WIN * N_PAT], BF16, kind="ExternalInput")
    csum_in = nc.dram_tensor("csum", [N_PAT, DK], BF16, kind="ExternalInput")
    idb_in = nc.dram_tensor("idb", [128, 128], BF16, kind="ExternalInput")
    ones81_in = nc.dram_tensor("ones81", [N_PAT, 1], BF16, kind="ExternalInput")
    out_d = nc.dram_tensor("out", [Bs, T, N_NODES, DK], F32, kind="ExternalOutput")

    x_flat = x_in.rearrange("b t n d -> b (t n) d")
    out_flat = out_d.rearrange("b t n d -> b (t n) d")

    with tile.TileContext(nc) as tc:
        with (
            tc.tile_pool(name="consts", bufs=1) as cpool,
            tc.tile_pool(name="stream", bufs=3) as spool,
            tc.tile_pool(name="ut", bufs=1) as utpool,
            tc.tile_pool(name="psL", bufs=3, space="PSUM") as psL,
            tc.tile_pool(name="psP", bufs=2, space="PSUM") as psP,
            tc.tile_pool(name="psA", bufs=2, space="PSUM") as psA,
            tc.tile_pool(name="psV", bufs=1, space="PSUM") as psV,
        ):
            # ---- constants into SBUF ----
            wt_sb = {}
            b_sb = {}
            for k in ("q", "k", "v", "u"):
                wt_sb[k] = cpool.tile([128, 2, DK], BF16, tag=f"wt{k}", name=f"wt{k}_sb")
                nc.sync.dma_start(out=wt_sb[k], in_=wts[k].rearrange("c d m -> d c m"))
                b_sb[k] = cpool.tile([DK, 1], F32, tag=f"b{k}", name=f"b{k}_sb")
                nc.sync.dma_start(out=b_sb[k], in_=biases_in[k][:, :])
            mT_sb = cpool.tile([DK, S_WIN * N_PAT], BF16, tag="mT")
            nc.sync.dma_start(out=mT_sb, in_=mT_in[:, :])
            csum_sb = cpool.tile([N_PAT, DK], BF16, tag="csum")
            nc.sync.dma_start(out=csum_sb, in_=csum_in[:, :])
            idb_sb = cpool.tile([128, 128], BF16, tag="idb")
            nc.sync.dma_start(out=idb_sb, in_=idb_in[:, :])
            ones81_sb = cpool.tile([N_PAT, 1], BF16, tag="ones81")
            nc.sync.dma_start(out=ones81_sb, in_=ones81_in[:, :])

            # Absorb const-DMA semaphores into dedicated PE transposes:
            # walrus's self-loading matmul allows at most 2 sync waits, so
            # real matmuls must never be the first reader of a const DMA.
            def absorb(t):
                p, f = t.shape[0], int(np.prod(t.shape[1:]))
                scr = psV.tile([128, 4, 132], BF16, tag="vx", name="absorb_scr")
                nc.tensor.transpose(
                    out=scr[0:f, 0, 0:p],
                    in_=t,
                    identity=idb_sb[0:p, 0:p],
                )

            for k in ("q", "k", "v", "u"):
                for cd in range(2):
                    absorb(wt_sb[k][:, cd, :])
            absorb(mT_sb)
            absorb(csum_sb)
            absorb(ones81_sb)
            absorb(idb_sb)

            # Pre-zeroed attention-weight ring: exp writes only the diagonal
            # 64x64 blocks, so the off-diagonal blocks stay zero and ONE
            # K=128 block-diagonal AV matmul per pair replaces two K=64 ones.
            attn_ring = []
            for zi in range(3):
                az = cpool.tile([128, 128], BF16, tag=f"az{zi}", name=f"attn_z{zi}")
                nc.vector.memset(az, 0.0)
                attn_ring.append(az)

            for b in range(Bs):
                ut = utpool.tile([128, TOK], BF16, tag="ut")
                for c in range(nsl):
                    tok0 = c * SL
                    # ---- DMA-transposed load: xt chunks [128 d, 512 tok] ----
                    xt = []
                    for cd in range(2):
                        xt_c = spool.tile([128, SL], BF16, tag=f"xt{cd}")
                        nc.sync.dma_start_transpose(
                            out=xt_c,
                            in_=x_flat[
                                b, tok0 : tok0 + SL, cd * 128 : (cd + 1) * 128
                            ],
                        )
                        xt.append(xt_c)

                    def linear(key):
                        ps = psL.tile([128, SL], F32, tag="lin", name=f"{key}_ps")
                        for cd in range(2):
                            nc.tensor.matmul(
                                ps,
                                lhsT=wt_sb[key][:, cd, :],
                                rhs=xt[cd],
                                start=(cd == 0),
                                stop=(cd == 1),
                            )
                        return ps

                    # ---- u linear -> UT[,:tok] (bf16, +bias) ----
                    u_ps = linear("u")
                    nc.scalar.activation(
                        out=ut[:, tok0 : tok0 + SL],
                        in_=u_ps,
                        func=AF.Identity,
                        bias=b_sb["u"],
                    )

                    # ---- pattern pipeline ----
                    rd4 = None
                    nt_bf = None
                    if c >= 1:
                        j0 = 128 if c == 1 else 0
                        nsim = SL - j0
                        sim_ps = psP.tile([N_PAT, SL], F32, tag="pat", name="sim_ps")
                        for s in range(S_WIN):
                            ucol = tok0 + j0 - INJ0 + 64 * s
                            nc.tensor.matmul(
                                sim_ps[:, j0:],
                                lhsT=mT_sb[:, s * N_PAT : (s + 1) * N_PAT],
                                rhs=ut[:, ucol : ucol + nsim],
                                start=(s == 0),
                                stop=(s == S_WIN - 1),
                            )
                        e_t = spool.tile([N_PAT, SL], BF16, tag="e")
                        if j0 > 0:
                            nc.vector.memset(e_t[:, 0:j0], 0.0)
                        nc.scalar.activation(
                            out=e_t[:, j0:], in_=sim_ps[:, j0:], func=AF.Exp
                        )
                        # unnormalized injection N = csum^T @ e  [128 d, 512]
                        n_ps = psP.tile([128, SL], F32, tag="pat", name="n_ps")
                        nc.tensor.matmul(
                            n_ps, lhsT=csum_sb, rhs=e_t, start=True, stop=True
                        )
                        nt_bf = spool.tile([128, SL], BF16, tag="ntbf")
                        nc.vector.tensor_copy(out=nt_bf, in_=n_ps)
                        # transposed denominator denT[tok] per 128-chunk
                        den4_ps = psP.tile([128, 4], F32, tag="pat", name="den4_ps")
                        for ch in range(4):
                            nc.tensor.matmul(
                                den4_ps[:, ch : ch + 1],
                                lhsT=e_t[:, ch * 128 : (ch + 1) * 128],
                                rhs=ones81_sb,
                                start=True,
                                stop=True,
                            )
                        rd4 = spool.tile([128, 4], F32, tag="rd4")
                        ch0 = j0 // 128
                        if ch0 > 0:
                            nc.vector.memset(rd4[:, 0:ch0], 0.0)
                        nc.vector.reciprocal(
                            out=rd4[:, ch0:], in_=den4_ps[:, ch0:]
                        )

                    # ---- K / Q linears -> bf16. K's bias is dropped: it adds
                    # a per-query constant to scores, invariant under softmax.
                    k_ps = linear("k")
                    kt_bf = spool.tile([128, SL], BF16, tag="kt")
                    nc.vector.tensor_copy(out=kt_bf, in_=k_ps)
                    q_ps = linear("q")
                    qt_bf = spool.tile([128, SL], BF16, tag="qt")
                    nc.scalar.activation(
                        out=qt_bf, in_=q_ps, func=AF.Identity, bias=b_sb["q"]
                    )

                    # ---- V linear -> bf16 VT -> transpose to V natural ----
                    v_ps = linear("v")
                    vt_bf = spool.tile([128, SL], BF16, tag="vt")
                    nc.scalar.activation(
                        out=vt_bf, in_=v_ps, func=AF.Identity, bias=b_sb["v"]
                    )
                    # vext[tok, (pr, d|1)]: pair pr rows 0:64 = even t, 64:128 odd
                    vx_ps = psV.tile([128, 4, 132], BF16, tag="vx", name="vx_ps")
                    for pr in range(4):
                        nc.tensor.transpose(
                            out=vx_ps[:, pr, 0:128],
                            in_=vt_bf[:, pr * 128 : (pr + 1) * 128],
                            identity=idb_sb,
                        )
                    vext = spool.tile([128, 4, 132], BF16, tag="vnat")
                    nc.scalar.copy(out=vext[:, :, 0:128], in_=vx_ps[:, :, 0:128])
                    nc.vector.memset(vext[:, :, 128:129], 1.0)

                    # ---- attention: 4 pairs of timesteps, scores [k, q] ----
                    # att tile regions: [0:128]=sc1, [128:256]=sc2, [256:385]=o|den
                    out_sb = spool.tile([128, 4, DK], F32, tag="osb")
                    for pr in range(4):
                        c1 = pr * 128
                        att = psA.tile([128, 512], F32, tag="att", name="att")
                        nc.tensor.matmul(
                            att[:, 0:128],
                            lhsT=kt_bf[:, c1 : c1 + 128],
                            rhs=qt_bf[:, c1 : c1 + 128],
                            start=True,
                            stop=True,
                        )
                        attn_bf = attn_ring[(c * 4 + pr) % 3]
                        if rd4 is not None:
                            nc.tensor.matmul(
                                att[:, 128:256],
                                lhsT=nt_bf[:, c1 : c1 + 128],
                                rhs=qt_bf[:, c1 : c1 + 128],
                                start=True,
                                stop=True,
                            )
                            sc2s = spool.tile([128, 128], BF16, tag="sc2s")
                            nc.vector.tensor_scalar_mul(
                                out=sc2s,
                                in0=att[:, 128:256],
                                scalar1=rd4[:, pr : pr + 1],
                            )
                            sccmb = spool.tile([128, 128], BF16, tag="sccmb")
                            nc.vector.tensor_tensor(
                                out=sccmb,
                                in0=att[:, 0:128],
                                in1=sc2s,
                                op=ALU.add,
                            )
                            for h in range(2):
                                r0 = 64 * h
                                nc.scalar.activation(
                                    out=attn_bf[r0 : r0 + 64, r0 : r0 + 64],
                                    in_=sccmb[r0 : r0 + 64, r0 : r0 + 64],
                                    func=AF.Exp,
                                    scale=scale,
                                )
                        else:
                            for h in range(2):
                                r0 = 64 * h
                                nc.scalar.activation(
                                    out=attn_bf[r0 : r0 + 64, r0 : r0 + 64],
                                    in_=att[r0 : r0 + 64, r0 : r0 + 64],
                                    func=AF.Exp,
                                    scale=scale,
                                )
                        nc.tensor.matmul(
                            att[:, 256:385],
                            lhsT=attn_bf,
                            rhs=vext[:, pr, 0:129],
                            start=True,
                            stop=True,
                        )
                        rs = spool.tile([128, 1], F32, tag="rs")
                        nc.vector.reciprocal(out=rs, in_=att[:, 384:385])
                        nc.vector.tensor_scalar_mul(
                            out=out_sb[:, pr, :], in0=att[:, 256:384], scalar1=rs
                        )

                    nc.sync.dma_start(
                        out=out_flat[b, tok0 : tok0 + SL, :].rearrange(
                            "(j p) d -> p j d", p=128
                        ),
                        in_=out_sb,
                    )
    nc.finalize()
    return nc


def _host_prep(inputs: dict) -> dict:
    f = np.float32
    bf = ml_dtypes.bfloat16
    aux = {}
    for k, (W, bias) in {
        "q": (inputs["WQ"], inputs["bQ"]),
        "k": (inputs["WK"], inputs["bK"]),
        "v": (inputs["WV"], inputs["bV"]),
        "u": (inputs["Wu"], inputs["bu"]),
    }.items():
        aux[f"wt{k}"] = np.ascontiguousarray(
            np.asarray(W, f).T.reshape(2, 128, DK)
        ).astype(bf)
        aux[f"b{k}"] = np.ascontiguousarray(np.asarray(bias, f).reshape(DK, 1))
    patterns = np.asarray(inputs["patterns"], f)
    m = patterns @ np.asarray(inputs["Wm"], f).T + np.asarray(inputs["bm"], f)
    aux["mT"] = np.ascontiguousarray(
        m.transpose(2, 1, 0).reshape(DK, S_WIN * N_PAT)
    ).astype(bf)
    aux["csum"] = np.ascontiguousarray(
        (patterns @ np.asarray(inputs["Wc"], f).T + np.asarray(inputs["bc"], f)).sum(
            axis=1
        )
    ).astype(bf)
    aux["idb"] = np.eye(128, dtype=bf)
    aux["ones81"] = np.ones([N_PAT, 1], bf)
    return aux


TRACE = False
LAST_RESULTS = None


def kernel(**inputs) -> np.ndarray:
    global LAST_RESULTS
    from concourse.bass_utils import run_bass_kernel_spmd

    x = np.asarray(inputs["x"], np.float32)
    B, T = x.shape[0], x.shape[1]
    bs = B // N_CORES
    x_bf = x.astype(ml_dtypes.bfloat16)
    aux = _host_prep(inputs)
    nc = build_program(bs, T)
    in_maps = [dict(aux, x=x_bf[i * bs : (i + 1) * bs]) for i in range(N_CORES)]
    res = run_bass_kernel_spmd(nc, in_maps, list(range(N_CORES)), trace=TRACE)
    LAST_RESULTS = res
    return np.concatenate([r["out"] for r in res.results], axis=0)



# revision 18
# speedup vs baseline: 1.0672x; 1.0672x over previous
"""DelayAttention Trainium2 kernel (v5).

Data-parallel over batch: B=16 split as 2 batches per core across 8 cores.
The two batches are interleaved slice-by-slice (512 tokens each) to fill
dependency bubbles.

v6 changes over v3 (502us baseline):
  1. Per-pair DVE work batched 2 pairs per instruction through strided PSUM
     views + stride-0 broadcast scalars: one scores-PSUM bank holds sc1/sc2
     for two pairs; rd4-scale, sc1+sc2s combine, rowsum-reciprocal, and the
     output eviction each run once per pair-group instead of once per pair.
  2. Exp batched per pair-group: the attn ring tiles are [128, 2, 128] and
     a [64, 2, 64] AP exps both pairs' same-height diagonal blocks in one
     Act instruction (4 exps/slice instead of 8).
  3. The two batches interleave slice-by-slice to fill dependency bubbles.
  4. V bias on host (softmax rows sum to 1: out = o_unb/rs + bV); K bias
     dropped (softmax-invariant).
  5. PSUM: psL 2 (linears), psP 2 (pattern), psS 2 (scores 2 pairs/bank),
     psAV 1 (AV out 2 pairs/bank), psV 1 (V transposes).
  NOTE (learned on HW): PE col-tiling (tile_position[1]=64) and SBUF->SBUF
  DMA XBAR transposes both produce garbage on real hardware -- avoid.
"""

import os
import sys

import numpy as np

for _p in ("/opt/trn_rl_repo",):
    if _p not in sys.path and os.path.isdir(_p):
        sys.path.insert(0, _p)

import ml_dtypes  # noqa: E402

import concourse.bass as bass  # noqa: E402
import concourse.mybir as mybir  # noqa: E402
import concourse.tile as tile  # noqa: E402
from concourse import bacc  # noqa: E402

F32 = mybir.dt.float32
BF16 = mybir.dt.bfloat16
AX = mybir.AxisListType.X
AF = mybir.ActivationFunctionType
ALU = mybir.AluOpType

N_CORES = 8
N_NODES = 64          # N
D_MODEL = 256         # D
DK = 128
S_WIN = 10            # window size
N_PAT = 8             # patterns
SL = 512              # tokens per slice
INJ0 = S_WIN * N_NODES  # 640: first injected token


def build_program(Bs: int, T: int) -> bass.Bass:
    TOK = T * N_NODES
    nsl = TOK // SL
    assert TOK % SL == 0
    scale = 1.0 / float(np.sqrt(DK))

    nc = bacc.Bacc("TRN2", target_bir_lowering=False, debug=False)

    x_in = nc.dram_tensor("x", [Bs, T, N_NODES, D_MODEL], BF16, kind="ExternalInput")
    wts = {
        k: nc.dram_tensor(f"wt{k}", [2, 128, DK], BF16, kind="ExternalInput")
        for k in ("q", "k", "v", "u")
    }
    biases_in = {
        k: nc.dram_tensor(f"b{k}", [DK, 1], F32, kind="ExternalInput")
        for k in ("q", "u")
    }
    mT_in = nc.dram_tensor("mT", [DK, S_WIN * N_PAT], BF16, kind="ExternalInput")
    csum_in = nc.dram_tensor("csum", [N_PAT, DK], BF16, kind="ExternalInput")
    idb_in = nc.dram_tensor("idb", [128, 128], BF16, kind="ExternalInput")
    ones81_in = nc.dram_tensor("ones81", [N_PAT, 1], BF16, kind="ExternalInput")
    out_d = nc.dram_tensor("out", [Bs, T, N_NODES, DK], F32, kind="ExternalOutput")

    x_flat = x_in.rearrange("b t n d -> b (t n) d")
    out_flat = out_d.rearrange("b t n d -> b (t n) d")

    with tile.TileContext(nc) as tc:
        with (
            tc.tile_pool(name="consts", bufs=1) as cpool,
            tc.tile_pool(name="stream", bufs=3) as spool,
            tc.tile_pool(name="ut", bufs=1) as utpool,
            tc.tile_pool(name="psL", bufs=2, space="PSUM") as psL,
            tc.tile_pool(name="psP", bufs=2, space="PSUM") as psP,
            tc.tile_pool(name="psS", bufs=2, space="PSUM") as psS,
            tc.tile_pool(name="psAV", bufs=1, space="PSUM") as psAV,
            tc.tile_pool(name="psV", bufs=1, space="PSUM") as psV,
        ):
            # ---- constants into SBUF ----
            wt_sb = {}
            b_sb = {}
            for k in ("q", "k", "v", "u"):
                wt_sb[k] = cpool.tile([128, 2, DK], BF16, tag=f"wt{k}", name=f"wt{k}_sb")
                nc.sync.dma_start(out=wt_sb[k], in_=wts[k].rearrange("c d m -> d c m"))
            for k in ("q", "u"):
                b_sb[k] = cpool.tile([DK, 1], F32, tag=f"b{k}", name=f"b{k}_sb")
                nc.sync.dma_start(out=b_sb[k], in_=biases_in[k][:, :])
            mT_sb = cpool.tile([DK, S_WIN * N_PAT], BF16, tag="mT")
            nc.sync.dma_start(out=mT_sb, in_=mT_in[:, :])
            csum_sb = cpool.tile([N_PAT, DK], BF16, tag="csum")
            nc.sync.dma_start(out=csum_sb, in_=csum_in[:, :])
            idb_sb = cpool.tile([128, 128], BF16, tag="idb")
            nc.sync.dma_start(out=idb_sb, in_=idb_in[:, :])
            ones81_sb = cpool.tile([N_PAT, 1], BF16, tag="ones81")
            nc.sync.dma_start(out=ones81_sb, in_=ones81_in[:, :])

            # Absorb const-DMA semaphores into dedicated PE transposes:
            # walrus's self-loading matmul allows at most 2 sync waits, so
            # real matmuls must never be the first reader of a const DMA.
            def absorb(t):
                p, f = t.shape[0], int(np.prod(t.shape[1:]))
                scr = psP.tile([128, 132], BF16, tag="pat", name="absorb_scr")
                nc.tensor.transpose(
                    out=scr[0:f, 0:p],
                    in_=t,
                    identity=idb_sb[0:p, 0:p],
                )

            for k in ("q", "k", "v", "u"):
                for cd in range(2):
                    absorb(wt_sb[k][:, cd, :])
            absorb(mT_sb)
            absorb(csum_sb)
            absorb(ones81_sb)
            absorb(idb_sb)

            # Pre-zeroed attention-weight ring (one tile per PAIR-GROUP of 2
            # pairs): exp writes only the diagonal 64x64 blocks of each
            # [128, 128] plane, so the off-diagonal blocks stay zero and ONE
            # K=128 block-diagonal AV matmul per pair replaces two K=64 ones.
            # The [64, 2, 64] exp AP covers both planes' same-height diagonal
            # blocks in a single Act instruction.
            attn_ring = []
            for zi in range(3):
                az = cpool.tile([128, 2, 128], BF16, tag=f"az{zi}", name=f"attn_z{zi}")
                nc.vector.memset(az, 0.0)
                attn_ring.append(az)

            ut = [
                utpool.tile([128, TOK], BF16, tag=f"ut{b}", name=f"ut{b}")
                for b in range(Bs)
            ]

            for c in range(nsl):
                for b in range(Bs):
                    tok0 = c * SL
                    # ---- DMA-transposed load: xt chunks [128 d, 512 tok] ----
                    xt = []
                    for cd in range(2):
                        xt_c = spool.tile([128, SL], BF16, tag=f"xt{cd}")
                        nc.sync.dma_start_transpose(
                            out=xt_c,
                            in_=x_flat[
                                b, tok0 : tok0 + SL, cd * 128 : (cd + 1) * 128
                            ],
                        )
                        xt.append(xt_c)

                    def linear(key):
                        ps = psL.tile([128, SL], F32, tag="lin", name=f"{key}_ps")
                        for cd in range(2):
                            nc.tensor.matmul(
                                ps,
                                lhsT=wt_sb[key][:, cd, :],
                                rhs=xt[cd],
                                start=(cd == 0),
                                stop=(cd == 1),
                            )
                        return ps

                    # ---- u linear -> UT[,:tok] (bf16, +bias) ----
                    u_ps = linear("u")
                    nc.scalar.activation(
                        out=ut[b][:, tok0 : tok0 + SL],
                        in_=u_ps,
                        func=AF.Identity,
                        bias=b_sb["u"],
                    )

                    # ---- K / Q / V linears. K bias dropped (softmax-
                    # invariant); V bias added on host.
                    k_ps = linear("k")
                    kt_bf = spool.tile([128, SL], BF16, tag="kt")
                    nc.vector.tensor_copy(out=kt_bf, in_=k_ps)
                    q_ps = linear("q")
                    qt_bf = spool.tile([128, SL], BF16, tag="qt")
                    nc.scalar.activation(
                        out=qt_bf, in_=q_ps, func=AF.Identity, bias=b_sb["q"]
                    )
                    # ---- V natural [tok, dk] directly: lhsT = xt token-
                    # block (stationary), rhs = Wv^T chunk (moving). Rows
                    # 0:64 of each pr = even timestep of the pair, 64:128
                    # odd. No separate linear-transposed V or PE transposes.
                    vnat_ps = psV.tile([128, 4, 128], F32, tag="vx", name="vnat_ps")
                    for g in range(4):
                        for cd in range(2):
                            nc.tensor.matmul(
                                vnat_ps[:, g, :],
                                lhsT=xt[cd][:, g * 128 : (g + 1) * 128],
                                rhs=wt_sb["v"][:, cd, :],
                                start=(cd == 0),
                                stop=(cd == 1),
                            )
                    vext = spool.tile([128, 4, 132], BF16, tag="vnat")
                    nc.vector.memset(vext[:, :, 128:129], 1.0)
                    nc.scalar.copy(out=vext[:, :, 0:128], in_=vnat_ps)

                    # ---- pattern pipeline ----
                    rd4 = None
                    nt_bf = None
                    if c >= 1:
                        j0 = 128 if c == 1 else 0
                        nsim = SL - j0
                        sim_ps = psP.tile([N_PAT, SL], F32, tag="pat", name="sim_ps")
                        for s in range(S_WIN):
                            ucol = tok0 + j0 - INJ0 + 64 * s
                            nc.tensor.matmul(
                                sim_ps[:, j0:],
                                lhsT=mT_sb[:, s * N_PAT : (s + 1) * N_PAT],
                                rhs=ut[b][:, ucol : ucol + nsim],
                                start=(s == 0),
                                stop=(s == S_WIN - 1),
                            )
                        e_t = spool.tile([N_PAT, SL], BF16, tag="e")
                        if j0 > 0:
                            nc.vector.memset(e_t[:, 0:j0], 0.0)
                        nc.scalar.activation(
                            out=e_t[:, j0:], in_=sim_ps[:, j0:], func=AF.Exp
                        )
                        # unnormalized injection N = csum^T @ e  [128 d, 512]
                        n_ps = psP.tile([128, SL], F32, tag="pat", name="n_ps")
                        nc.tensor.matmul(
                            n_ps, lhsT=csum_sb, rhs=e_t, start=True, stop=True
                        )
                        nt_bf = spool.tile([128, SL], BF16, tag="ntbf")
                        nc.vector.tensor_copy(out=nt_bf, in_=n_ps)
                        # transposed denominator denT[tok] per 128-chunk
                        den4_ps = psP.tile([128, 4], F32, tag="pat", name="den4_ps")
                        for ch in range(4):
                            nc.tensor.matmul(
                                den4_ps[:, ch : ch + 1],
                                lhsT=e_t[:, ch * 128 : (ch + 1) * 128],
                                rhs=ones81_sb,
                                start=True,
                                stop=True,
                            )
                        rd4 = spool.tile([128, 4], F32, tag="rd4")
                        ch0 = j0 // 128
                        if ch0 > 0:
                            nc.vector.memset(rd4[:, 0:ch0], 0.0)
                        nc.vector.reciprocal(
                            out=rd4[:, ch0:], in_=den4_ps[:, ch0:]
                        )

                    # ---- attention: pairs of timesteps, 2 pairs per
                    # scores bank. att regions per sub-pair jj:
                    # [256jj : 256jj+128]=sc1, [256jj+128 : 256jj+256]=sc2.
                    out_sb = spool.tile([128, 4, DK], F32, tag="osb")
                    for j in range(2):
                        att = psS.tile([128, 512], F32, tag="att", name="att")
                        for jj in range(2):
                            pr = 2 * j + jj
                            c1 = pr * 128
                            o = 256 * jj
                            nc.tensor.matmul(
                                att[:, o : o + 128],
                                lhsT=kt_bf[:, c1 : c1 + 128],
                                rhs=qt_bf[:, c1 : c1 + 128],
                                start=True,
                                stop=True,
                            )
                            if rd4 is not None:
                                nc.tensor.matmul(
                                    att[:, o + 128 : o + 256],
                                    lhsT=nt_bf[:, c1 : c1 + 128],
                                    rhs=qt_bf[:, c1 : c1 + 128],
                                    start=True,
                                    stop=True,
                                )
                        att3 = att.rearrange("p (g x) -> p g x", x=256)
                        sc1_v = att3[:, :, 0:128]
                        if rd4 is not None:
                            sc2_v = att3[:, :, 128:256]
                            rdb = (
                                rd4[:, 2 * j : 2 * j + 2]
                                .unsqueeze(2)
                                .broadcast_to([128, 2, 128])
                            )
                            sc2s = spool.tile([128, 2, 128], BF16, tag="sc2s")
                            nc.vector.tensor_tensor(
                                out=sc2s, in0=sc2_v, in1=rdb, op=ALU.mult
                            )
                            sccmb = spool.tile([128, 2, 128], BF16, tag="sccmb")
                            nc.vector.tensor_tensor(
                                out=sccmb, in0=sc1_v, in1=sc2s, op=ALU.add
                            )

                        av = psAV.tile([128, 2, 132], F32, tag="av", name="av")
                        attn_bf = attn_ring[((c * Bs + b) * 2 + j) % 3]
                        for h in range(2):
                            r0 = 64 * h
                            if rd4 is not None:
                                src = sccmb[r0 : r0 + 64, :, r0 : r0 + 64]
                            else:
                                src = att3[r0 : r0 + 64, :, r0 : r0 + 64]
                            nc.scalar.activation(
                                out=attn_bf[r0 : r0 + 64, :, r0 : r0 + 64],
                                in_=src,
                                func=AF.Exp,
                                scale=scale,
                            )
                        for jj in range(2):
                            pr = 2 * j + jj
                            nc.tensor.matmul(
                                av[:, jj, 0:129],
                                lhsT=attn_bf[:, jj, :],
                                rhs=vext[:, pr, 0:129],
                                start=True,
                                stop=True,
                            )
                        rs2 = spool.tile([128, 2, 1], F32, tag="rs2")
                        nc.vector.reciprocal(out=rs2, in_=av[:, :, 128:129])
                        nc.vector.tensor_tensor(
                            out=out_sb[:, 2 * j : 2 * j + 2, :],
                            in0=av[:, :, 0:128],
                            in1=rs2.broadcast_to([128, 2, 128]),
                            op=ALU.mult,
                        )

                    nc.sync.dma_start(
                        out=out_flat[b, tok0 : tok0 + SL, :].rearrange(
                            "(j p) d -> p j d", p=128
                        ),
                        in_=out_sb,
                    )
    nc.finalize()
    return nc


def _host_prep(inputs: dict) -> dict:
    f = np.float32
    bf = ml_dtypes.bfloat16
    aux = {}
    for k, (W, bias) in {
        "q": (inputs["WQ"], inputs["bQ"]),
        "k": (inputs["WK"], None),
        "v": (inputs["WV"], None),
        "u": (inputs["Wu"], inputs["bu"]),
    }.items():
        aux[f"wt{k}"] = np.ascontiguousarray(
            np.asarray(W, f).T.reshape(2, 128, DK)
        ).astype(bf)
        if bias is not None:
            aux[f"b{k}"] = np.ascontiguousarray(np.asarray(bias, f).reshape(DK, 1))
    patterns = np.asarray(inputs["patterns"], f)
    m = patterns @ np.asarray(inputs["Wm"], f).T + np.asarray(inputs["bm"], f)
    aux["mT"] = np.ascontiguousarray(
        m.transpose(2, 1, 0).reshape(DK, S_WIN * N_PAT)
    ).astype(bf)
    aux["csum"] = np.ascontiguousarray(
        (patterns @ np.asarray(inputs["Wc"], f).T + np.asarray(inputs["bc"], f)).sum(
            axis=1
        )
    ).astype(bf)
    aux["idb"] = np.eye(128, dtype=bf)
    aux["ones81"] = np.ones([N_PAT, 1], bf)
    return aux


TRACE = False
LAST_RESULTS = None


def kernel(**inputs) -> np.ndarray:
    global LAST_RESULTS
    from concourse.bass_utils import run_bass_kernel_spmd

    x = np.asarray(inputs["x"], np.float32)
    B, T = x.shape[0], x.shape[1]
    bs = B // N_CORES
    x_bf = x.astype(ml_dtypes.bfloat16)
    aux = _host_prep(inputs)
    nc = build_program(bs, T)
    in_maps = [dict(aux, x=x_bf[i * bs : (i + 1) * bs]) for i in range(N_CORES)]
    res = run_bass_kernel_spmd(nc, in_maps, list(range(N_CORES)), trace=TRACE)
    LAST_RESULTS = res
    bV = np.asarray(inputs["bV"], np.float32)
    out = np.concatenate([r["out"] for r in res.results], axis=0)
    return (out + bV).astype(np.float32)


# revision 22
# speedup vs baseline: 1.0791x; 1.0112x over previous
"""DelayAttention Trainium2 kernel (v5).

Data-parallel over batch: B=16 split as 2 batches per core across 8 cores.
The two batches are interleaved slice-by-slice (512 tokens each) to fill
dependency bubbles.

v6 changes over v3 (502us baseline):
  1. Per-pair DVE work batched 2 pairs per instruction through strided PSUM
     views + stride-0 broadcast scalars: one scores-PSUM bank holds sc1/sc2
     for two pairs; rd4-scale, sc1+sc2s combine, rowsum-reciprocal, and the
     output eviction each run once per pair-group instead of once per pair.
  2. Exp batched per pair-group: the attn ring tiles are [128, 2, 128] and
     a [64, 2, 64] AP exps both pairs' same-height diagonal blocks in one
     Act instruction (4 exps/slice instead of 8).
  3. The two batches interleave slice-by-slice, and attention is software-
     pipelined one iteration behind: iteration i runs [linears, V-natural,
     sim, AV(i-1) + eviction, sc1(i), inject, den4, sc2(i), combines,
     exps(i)], so the PE never sits in the sim->exp->inject->cast->sc2
     dependency chain -- AV/sc1 of the pipelined iterations fill the gap.
  4. V bias on host (softmax rows sum to 1: out = o_unb/rs + bV); K bias
     dropped (softmax-invariant).
  5. PSUM: psL 2 (linears), psP 2 (pattern), psS 2 (scores 2 pairs/bank),
     psAV 1 (AV out 2 pairs/bank), psV 1 (V natural).
  NOTE (learned on HW): PE col-tiling (tile_position[1]=64) and SBUF->SBUF
  DMA XBAR transposes both produce garbage on real hardware -- avoid.
"""

import os
import sys

import numpy as np

for _p in ("/opt/trn_rl_repo",):
    if _p not in sys.path and os.path.isdir(_p):
        sys.path.insert(0, _p)

import ml_dtypes  # noqa: E402

import concourse.bass as bass  # noqa: E402
import concourse.mybir as mybir  # noqa: E402
import concourse.tile as tile  # noqa: E402
from concourse import bacc  # noqa: E402

F32 = mybir.dt.float32
BF16 = mybir.dt.bfloat16
AX = mybir.AxisListType.X
AF = mybir.ActivationFunctionType
ALU = mybir.AluOpType

N_CORES = 8
N_NODES = 64          # N
D_MODEL = 256         # D
DK = 128
S_WIN = 10            # window size
N_PAT = 8             # patterns
SL = 512              # tokens per slice
INJ0 = S_WIN * N_NODES  # 640: first injected token


def build_program(Bs: int, T: int) -> bass.Bass:
    TOK = T * N_NODES
    nsl = TOK // SL
    assert TOK % SL == 0
    scale = 1.0 / float(np.sqrt(DK))

    nc = bacc.Bacc("TRN2", target_bir_lowering=False, debug=False)

    x_in = nc.dram_tensor("x", [Bs, T, N_NODES, D_MODEL], BF16, kind="ExternalInput")
    wts = {
        k: nc.dram_tensor(f"wt{k}", [2, 128, DK], BF16, kind="ExternalInput")
        for k in ("q", "k", "v", "u")
    }
    biases_in = {
        k: nc.dram_tensor(f"b{k}", [DK, 1], F32, kind="ExternalInput")
        for k in ("q", "u")
    }
    mT_in = nc.dram_tensor("mT", [DK, S_WIN * N_PAT], BF16, kind="ExternalInput")
    csum_in = nc.dram_tensor("csum", [N_PAT, DK], BF16, kind="ExternalInput")
    idb_in = nc.dram_tensor("idb", [128, 128], BF16, kind="ExternalInput")
    ones81_in = nc.dram_tensor("ones81", [N_PAT, 1], BF16, kind="ExternalInput")
    out_d = nc.dram_tensor("out", [Bs, T, N_NODES, DK], F32, kind="ExternalOutput")

    x_flat = x_in.rearrange("b t n d -> b (t n) d")
    out_flat = out_d.rearrange("b t n d -> b (t n) d")

    with tile.TileContext(nc) as tc:
        with (
            tc.tile_pool(name="consts", bufs=1) as cpool,
            tc.tile_pool(name="stream", bufs=3) as spool,
            tc.tile_pool(name="ut", bufs=1) as utpool,
            tc.tile_pool(name="psL", bufs=2, space="PSUM") as psL,
            tc.tile_pool(name="psP", bufs=2, space="PSUM") as psP,
            tc.tile_pool(name="psS", bufs=2, space="PSUM") as psS,
            tc.tile_pool(name="psAV", bufs=1, space="PSUM") as psAV,
            tc.tile_pool(name="psV", bufs=1, space="PSUM") as psV,
        ):
            # ---- constants into SBUF ----
            wt_sb = {}
            b_sb = {}
            for k in ("q", "k", "v", "u"):
                wt_sb[k] = cpool.tile([128, 2, DK], BF16, tag=f"wt{k}", name=f"wt{k}_sb")
                nc.sync.dma_start(out=wt_sb[k], in_=wts[k].rearrange("c d m -> d c m"))
            for k in ("q", "u"):
                b_sb[k] = cpool.tile([DK, 1], F32, tag=f"b{k}", name=f"b{k}_sb")
                nc.sync.dma_start(out=b_sb[k], in_=biases_in[k][:, :])
            mT_sb = cpool.tile([DK, S_WIN * N_PAT], BF16, tag="mT")
            nc.sync.dma_start(out=mT_sb, in_=mT_in[:, :])
            csum_sb = cpool.tile([N_PAT, DK], BF16, tag="csum")
            nc.sync.dma_start(out=csum_sb, in_=csum_in[:, :])
            idb_sb = cpool.tile([128, 128], BF16, tag="idb")
            nc.sync.dma_start(out=idb_sb, in_=idb_in[:, :])
            ones81_sb = cpool.tile([N_PAT, 1], BF16, tag="ones81")
            nc.sync.dma_start(out=ones81_sb, in_=ones81_in[:, :])

            # Absorb const-DMA semaphores into dedicated PE transposes:
            # walrus's self-loading matmul allows at most 2 sync waits, so
            # real matmuls must never be the first reader of a const DMA.
            def absorb(t):
                p, f = t.shape[0], int(np.prod(t.shape[1:]))
                scr = psP.tile([128, 132], BF16, tag="pat", name="absorb_scr")
                nc.tensor.transpose(
                    out=scr[0:f, 0:p],
                    in_=t,
                    identity=idb_sb[0:p, 0:p],
                )

            for k in ("q", "k", "v", "u"):
                for cd in range(2):
                    absorb(wt_sb[k][:, cd, :])
            absorb(mT_sb)
            absorb(csum_sb)
            absorb(ones81_sb)
            absorb(idb_sb)

            # Pre-zeroed attention-weight ring (one tile per PAIR-GROUP of 2
            # pairs): exp writes only the diagonal 64x64 blocks of each
            # [128, 128] plane, so the off-diagonal blocks stay zero and ONE
            # K=128 block-diagonal AV matmul per pair replaces two K=64 ones.
            # The [64, 2, 64] exp AP covers both planes' same-height diagonal
            # blocks in a single Act instruction.
            attn_ring = []
            for zi in range(4):
                az = cpool.tile([128, 2, 128], BF16, tag=f"az{zi}", name=f"attn_z{zi}")
                nc.vector.memset(az, 0.0)
                attn_ring.append(az)

            ut = [
                utpool.tile([128, TOK], BF16, tag=f"ut{b}", name=f"ut{b}")
                for b in range(Bs)
            ]

            # ---- stage B of the attention pipeline: AV + eviction + out
            # DMA for the iteration whose scores/exps were issued last.
            def attn_flush(prev):
                b, tok0, rings, vext = prev
                out_sb = spool.tile([128, 4, DK], F32, tag="osb")
                for j in range(2):
                    av = psAV.tile([128, 2, 132], F32, tag="av", name="av")
                    for jj in range(2):
                        pr = 2 * j + jj
                        nc.tensor.matmul(
                            av[:, jj, 0:129],
                            lhsT=rings[j][:, jj, :],
                            rhs=vext[:, pr, 0:129],
                            start=True,
                            stop=True,
                        )
                    rs2 = spool.tile([128, 2, 1], F32, tag="rs2")
                    nc.vector.reciprocal(out=rs2, in_=av[:, :, 128:129])
                    nc.vector.tensor_tensor(
                        out=out_sb[:, 2 * j : 2 * j + 2, :],
                        in0=av[:, :, 0:128],
                        in1=rs2.broadcast_to([128, 2, 128]),
                        op=ALU.mult,
                    )
                nc.sync.dma_start(
                    out=out_flat[b, tok0 : tok0 + SL, :].rearrange(
                        "(j p) d -> p j d", p=128
                    ),
                    in_=out_sb,
                )

            prev = None
            it = -1
            for c in range(nsl):
                for b in range(Bs):
                    it += 1
                    tok0 = c * SL
                    # ---- DMA-transposed load: xt chunks [128 d, 512 tok] ----
                    xt = []
                    for cd in range(2):
                        xt_c = spool.tile([128, SL], BF16, tag=f"xt{cd}")
                        nc.sync.dma_start_transpose(
                            out=xt_c,
                            in_=x_flat[
                                b, tok0 : tok0 + SL, cd * 128 : (cd + 1) * 128
                            ],
                        )
                        xt.append(xt_c)

                    def linear(key):
                        ps = psL.tile([128, SL], F32, tag="lin", name=f"{key}_ps")
                        for cd in range(2):
                            nc.tensor.matmul(
                                ps,
                                lhsT=wt_sb[key][:, cd, :],
                                rhs=xt[cd],
                                start=(cd == 0),
                                stop=(cd == 1),
                            )
                        return ps

                    # ---- u linear -> UT[,:tok] (bf16, +bias) ----
                    u_ps = linear("u")
                    nc.scalar.activation(
                        out=ut[b][:, tok0 : tok0 + SL],
                        in_=u_ps,
                        func=AF.Identity,
                        bias=b_sb["u"],
                    )

                    # ---- K / Q / V linears. K bias dropped (softmax-
                    # invariant); V bias added on host.
                    k_ps = linear("k")
                    kt_bf = spool.tile([128, SL], BF16, tag="kt")
                    nc.vector.tensor_copy(out=kt_bf, in_=k_ps)
                    q_ps = linear("q")
                    qt_bf = spool.tile([128, SL], BF16, tag="qt")
                    nc.scalar.activation(
                        out=qt_bf, in_=q_ps, func=AF.Identity, bias=b_sb["q"]
                    )
                    # ---- V natural [tok, dk] directly: lhsT = xt token-
                    # block (stationary), rhs = Wv^T chunk (moving). Rows
                    # 0:64 of each pr = even timestep of the pair, 64:128
                    # odd. No separate linear-transposed V or PE transposes.
                    vnat_ps = psV.tile([128, 4, 128], F32, tag="vx", name="vnat_ps")
                    for g in range(4):
                        for cd in range(2):
                            nc.tensor.matmul(
                                vnat_ps[:, g, :],
                                lhsT=xt[cd][:, g * 128 : (g + 1) * 128],
                                rhs=wt_sb["v"][:, cd, :],
                                start=(cd == 0),
                                stop=(cd == 1),
                            )
                    vext = spool.tile([128, 4, 132], BF16, tag="vnat")
                    nc.vector.memset(vext[:, :, 128:129], 1.0)
                    nc.scalar.copy(out=vext[:, :, 0:128], in_=vnat_ps)

                    # ---- sim matmuls + e exp (Act waits on sim) ----
                    rd4 = None
                    nt_bf = None
                    e_t = None
                    j0 = 128 if c == 1 else 0
                    if c >= 1:
                        nsim = SL - j0
                        sim_ps = psP.tile([N_PAT, SL], F32, tag="pat", name="sim_ps")
                        for s in range(S_WIN):
                            ucol = tok0 + j0 - INJ0 + 64 * s
                            nc.tensor.matmul(
                                sim_ps[:, j0:],
                                lhsT=mT_sb[:, s * N_PAT : (s + 1) * N_PAT],
                                rhs=ut[b][:, ucol : ucol + nsim],
                                start=(s == 0),
                                stop=(s == S_WIN - 1),
                            )
                        e_t = spool.tile([N_PAT, SL], BF16, tag="e")
                        if j0 > 0:
                            nc.vector.memset(e_t[:, 0:j0], 0.0)
                        nc.scalar.activation(
                            out=e_t[:, j0:], in_=sim_ps[:, j0:], func=AF.Exp
                        )

                    # ---- AV + eviction of the previous iteration: fills
                    # the PE while e_t/nt/den4 of this iteration resolve.
                    if prev is not None:
                        attn_flush(prev)

                    # ---- sc1 for both pair-groups (needs only kt/qt) ----
                    atts = []
                    for j in range(2):
                        att = psS.tile([128, 512], F32, tag="att", name="att")
                        atts.append(att)
                        for jj in range(2):
                            c1 = (2 * j + jj) * 128
                            o = 256 * jj
                            nc.tensor.matmul(
                                att[:, o : o + 128],
                                lhsT=kt_bf[:, c1 : c1 + 128],
                                rhs=qt_bf[:, c1 : c1 + 128],
                                start=True,
                                stop=True,
                            )

                    if c >= 1:
                        # unnormalized injection N = csum^T @ e  [128 d, 512]
                        n_ps = psP.tile([128, SL], F32, tag="pat", name="n_ps")
                        nc.tensor.matmul(
                            n_ps, lhsT=csum_sb, rhs=e_t, start=True, stop=True
                        )
                        nt_bf = spool.tile([128, SL], BF16, tag="ntbf")
                        nc.vector.tensor_copy(out=nt_bf, in_=n_ps)
                        # transposed denominator denT[tok] per 128-chunk
                        den4_ps = psP.tile([128, 4], F32, tag="pat", name="den4_ps")
                        for ch in range(4):
                            nc.tensor.matmul(
                                den4_ps[:, ch : ch + 1],
                                lhsT=e_t[:, ch * 128 : (ch + 1) * 128],
                                rhs=ones81_sb,
                                start=True,
                                stop=True,
                            )
                        rd4 = spool.tile([128, 4], F32, tag="rd4")
                        ch0 = j0 // 128
                        if ch0 > 0:
                            nc.vector.memset(rd4[:, 0:ch0], 0.0)
                        nc.vector.reciprocal(
                            out=rd4[:, ch0:], in_=den4_ps[:, ch0:]
                        )

                    # ---- sc2, combine, exp per pair-group; AV runs next
                    # iteration via attn_flush. att regions per sub-pair jj:
                    # [256jj : 256jj+128]=sc1, [256jj+128 : 256jj+256]=sc2.
                    rings = []
                    for j in range(2):
                        att = atts[j]
                        att3 = att.rearrange("p (g x) -> p g x", x=256)
                        if rd4 is not None:
                            for jj in range(2):
                                c1 = (2 * j + jj) * 128
                                o = 256 * jj
                                nc.tensor.matmul(
                                    att[:, o + 128 : o + 256],
                                    lhsT=nt_bf[:, c1 : c1 + 128],
                                    rhs=qt_bf[:, c1 : c1 + 128],
                                    start=True,
                                    stop=True,
                                )
                            rdb = (
                                rd4[:, 2 * j : 2 * j + 2]
                                .unsqueeze(2)
                                .broadcast_to([128, 2, 128])
                            )
                            sc2s = spool.tile([128, 2, 128], BF16, tag="sc2s")
                            nc.vector.tensor_tensor(
                                out=sc2s, in0=att3[:, :, 128:256], in1=rdb,
                                op=ALU.mult,
                            )
                            sccmb = spool.tile([128, 2, 128], BF16, tag="sccmb")
                            nc.vector.tensor_tensor(
                                out=sccmb, in0=att3[:, :, 0:128], in1=sc2s,
                                op=ALU.add,
                            )
                        attn_bf = attn_ring[(it * 2 + j) % 4]
                        for h in range(2):
                            r0 = 64 * h
                            if rd4 is not None:
                                src = sccmb[r0 : r0 + 64, :, r0 : r0 + 64]
                            else:
                                src = att3[r0 : r0 + 64, :, r0 : r0 + 64]
                            nc.scalar.activation(
                                out=attn_bf[r0 : r0 + 64, :, r0 : r0 + 64],
                                in_=src,
                                func=AF.Exp,
                                scale=scale,
                            )
                        rings.append(attn_bf)
                    prev = (b, tok0, rings, vext)

            attn_flush(prev)
    nc.finalize()
    return nc


def _host_prep(inputs: dict) -> dict:
    f = np.float32
    bf = ml_dtypes.bfloat16
    aux = {}
    for k, (W, bias) in {
        "q": (inputs["WQ"], inputs["bQ"]),
        "k": (inputs["WK"], None),
        "v": (inputs["WV"], None),
        "u": (inputs["Wu"], inputs["bu"]),
    }.items():
        aux[f"wt{k}"] = np.ascontiguousarray(
            np.asarray(W, f).T.reshape(2, 128, DK)
        ).astype(bf)
        if bias is not None:
            aux[f"b{k}"] = np.ascontiguousarray(np.asarray(bias, f).reshape(DK, 1))
    patterns = np.asarray(inputs["patterns"], f)
    m = patterns @ np.asarray(inputs["Wm"], f).T + np.asarray(inputs["bm"], f)
    aux["mT"] = np.ascontiguousarray(
        m.transpose(2, 1, 0).reshape(DK, S_WIN * N_PAT)
    ).astype(bf)
    aux["csum"] = np.ascontiguousarray(
        (patterns @ np.asarray(inputs["Wc"], f).T + np.asarray(inputs["bc"], f)).sum(
            axis=1
        )
    ).astype(bf)
    aux["idb"] = np.eye(128, dtype=bf)
    aux["ones81"] = np.ones([N_PAT, 1], bf)
    return aux


TRACE = False
LAST_RESULTS = None


def kernel(**inputs) -> np.ndarray:
    global LAST_RESULTS
    from concourse.bass_utils import run_bass_kernel_spmd

    x = np.asarray(inputs["x"], np.float32)
    B, T = x.shape[0], x.shape[1]
    bs = B // N_CORES
    x_bf = x.astype(ml_dtypes.bfloat16)
    aux = _host_prep(inputs)
    nc = build_program(bs, T)
    in_maps = [dict(aux, x=x_bf[i * bs : (i + 1) * bs]) for i in range(N_CORES)]
    res = run_bass_kernel_spmd(nc, in_maps, list(range(N_CORES)), trace=TRACE)
    LAST_RESULTS = res
    bV = np.asarray(inputs["bV"], np.float32)
    out = np.concatenate([r["out"] for r in res.results], axis=0)
    return (out + bV).astype(np.float32)
